# revision 27
# baseline (speedup 1.0000x reference)
"""De-emphasis IIR filter y[n] = c*y[n-1] + x[n] (c=0.95) on 8 NeuronCores.

Input: (64, 524288) fp32. Pure data parallel: 8 rows per core.

The recurrence runs on the TensorEngine instead of the DVE scan (the
native tensor_tensor_scan runs at 0.5 elem/cycle/partition -> ~70us per
core of DVE time; the PE does the same work in ~34us and overlaps DMA).

Math: split each row into 4096 blocks of 128 samples. With n = 128j + p,

    y[128j + p] = sum_{s<=p} c^(p-s) x[128j+s]           (matrix L)
                + sum_s c^(p+128-s) x[128(j-1)+s]        (matrix M1)
                + O(c^(129+p))                            (truncated)

c^129 ~ 1.3e-3, so the dropped tail contributes ~4e-4 relative RMS --
far below the bf16 transport noise (~2.4e-3) and the 2e-2 gate. Each
128-sample output block is L @ x_j + M1 @ x_{j-1}: two accumulating
[128x128] bf16 matmuls into the same PSUM region, where the M1 pass
reads the SAME SBUF tile shifted one block-column left (rows carry 2
leading zero guard columns so block -1 reads zeros).

Layout: the host block-transposes each row to [128 partitions(=p), 4096
blocks(=j)] bf16, so sample 128j+p sits at [p, j]; all HBM traffic is
bf16. The device computes y in the same layout; the host transposes
back and upcasts.

Per core: exactly 8 HWDGE DMAs on the SP ring in pinned FIFO order
(w, x0..x3, y01, y2, y3) so no DMA carries a completion-lane-reuse
wait. PSUM pool: 4 tiles of [128, 1024] fp32 (2 banks each), 4 matmuls
per tile (L/M1 per 512-col half; moving-dim max 512), then one
[128,1024] eviction copy (fp32->bf16, PSUM->SBUF). Evictions for pairs
0,1 run on DVE and pairs 2,3 on ACT so each y-DMA waits on a single
engine's semaphore.

This walrus build allows ONE semaphore wait per instruction. bass pairs
every Matmult with its own Ldweights (which absorbs the weights-dep
wait), and the tile framework elides waits already observed by an
earlier DATAPATH instruction on the same engine (NoOps don't count).
So every PSUM-tile-reuse WAR wait (eviction on DVE/ACT) and every
pair-first x-DMA wait is pre-absorbed by a tiny explicit ldweights that
reads one column of the producing tile; the real matmuls then carry at
most the PE self-wait (PSUM WAW bookkeeping). A burst of dummy matmuls
at kernel start warms the PE HAM clock gate (1.2 -> 2.4 GHz) while the
first input DMA is in flight.
"""

import sys

import ml_dtypes
import numpy as np

if "/opt/trn_rl_repo" not in sys.path:
    sys.path.insert(0, "/opt/trn_rl_repo")

import concourse.bass as bass
import concourse.mybir as mybir
from concourse import tile
from concourse.tile import add_dep_helper
from concourse.bass_utils import run_bass_kernel_spmd

N_CORES = 8
BATCH = 64
T = 524288
P = 128               # SBUF partitions = samples per block
NBLK = T // P         # 4096 block-columns per row
G = 2                 # leading zero guard columns (block -1 for M1 pass)
SUB = 2               # rows per pair-tile
PAIRS = BATCH // N_CORES // SUB  # 4 pair-tiles per core
MM = 512              # matmul moving-dim max
PCH = 1024            # psum tile columns (2 banks)
PBUFS = 4             # psum tiles in rotation (4 x 2 banks = all 8)
COEFF = 0.95
N_WARM = 15           # dummy matmuls to warm the PE clock gate

LAST_EXEC_TIME_NS = None
_nc_cache = None

F32 = mybir.dt.float32
BF16 = mybir.dt.bfloat16


def _weights():
    """Host-side [128, 256] bf16: cols 0:128 = L^T, 128:256 = M1^T.

    matmul(out, lhsT, rhs) computes lhsT.T @ rhs, so lhsT[s, p] holds the
    coefficient of input-sample s for output-sample p.
    """
    s = np.arange(P)[:, None].astype(np.float64)
    p = np.arange(P)[None, :].astype(np.float64)
    lt = np.where(p >= s, COEFF ** (p - s), 0.0)
    m1t = COEFF ** (p + 128 - s)
    return np.concatenate([lt, m1t], axis=1).astype(ml_dtypes.bfloat16)


def build_nc(pairs=PAIRS, nblk=NBLK):
    nc = bass.Bass()
    x_d = nc.declare_dram_parameter("x", [pairs, SUB, P, G + nblk], BF16,
                                    isOutput=False)
    w_d = nc.declare_dram_parameter("w", [P, 2 * P], BF16, isOutput=False)
    y_d = nc.declare_dram_parameter("y", [pairs, SUB, P, nblk], BF16,
                                    isOutput=True)

    dma_chain = []

    def chain_dma(inst):
        if dma_chain:
            add_dep_helper(inst.ins, dma_chain[-1].ins, sync=False,
                           reason="pin SP DMA FIFO order")
        dma_chain.append(inst)
        return inst

    with tile.TileContext(nc) as tc:
        with (
            tc.tile_pool(name="consts", bufs=1) as cpool,
            tc.tile_pool(name="xin", bufs=4) as xpool,
            tc.tile_pool(name="yout", bufs=1) as ypool,
            tc.tile_pool(name="acc", bufs=PBUFS, space="PSUM") as ppool,
        ):
            w = cpool.tile([P, 2 * P], BF16)
            wl = w[:, 0:P]
            wm = w[:, P:2 * P]
            scratch = cpool.tile([P, 4], BF16)
            gscratch = cpool.tile([P, 2], BF16)

            x_tiles = [xpool.tile([P, SUB, G + nblk], BF16, name=f"xt{i}",
                                  tag="xt")
                       for i in range(pairs)]

            # First loads ride the (otherwise idle) GPSIMD SWDGE queue: it
            # issues right after the core barrier, its completion sems are
            # separate from the 8 HWDGE lanes, and it gets the weights +
            # pair-0 row 0 on chip several us before the SP ring warms up.
            w_dma = nc.gpsimd.dma_start(w[:], w_d[:])
            x0a = nc.gpsimd.dma_start(x_tiles[0][:, 0, :], x_d[0, 0])
            xin = [nc.sync.dma_start(x_tiles[0][:, 1, :], x_d[0, 1])]
            dma_chain.append(xin[0])
            for i in range(1, pairs):
                xin.append(chain_dma(nc.sync.dma_start(
                    x_tiles[i][:], x_d[i].rearrange("s p l -> p s l"))))

            y_tiles = [ypool.tile([P, SUB, nblk], BF16, name=f"yt{i}")
                       for i in range(pairs)]

            def y_region(i, s):
                return y_tiles[i][:, s, :]

            # y DMAs ride the ACT HWDGE ring (separate FIFO from the SP
            # ring carrying inputs): outputs overlap the input stream, and
            # since half the evictions run on ACT itself, each y-DMA's
            # ACT-side deps are satisfied by program order and only the
            # DVE semaphore needs an explicit (single) wait.
            yout = []
            ship_abs = []

            def ship(dram_ap, sbuf_tile, last_dve_out):
                # A tiny ACT copy observes the DVE eviction semaphore first,
                # so the DMA itself carries only its ACT self-wait (walrus
                # allows ONE wait per instruction). Disjoint scratch columns
                # avoid same-engine WAW waits between these touches.
                k = len(ship_abs)
                ship_abs.append(
                    nc.scalar.copy(scratch[:, k:k + 1], last_dve_out[:, 0:1]))
                yout.append(nc.scalar.dma_start(dram_ap, sbuf_tile))

            # PE HAM warmup: dummy matmuls into the first psum pool tile;
            # results are discarded (start=True passes overwrite banks).
            warm_pt = ppool.tile([P, PCH], F32, name="warm", tag="pt")
            for _ in range(N_WARM):
                nc.tensor.matmul(warm_pt[:, 0:P], wl, wl,
                                 start=True, stop=True)

            evs = []      # (eviction inst, sbuf output AP) per chunk tile
            last_mm = None
            gp_touch = None
            tidx = 0
            for i in range(pairs):
                x_t = x_tiles[i]
                for s in range(SUB):
                    # Absorb this row's x-DMA wait on a ldweights so the
                    # row-first matmul doesn't carry it (its slot is needed
                    # for the PE self-wait). Pair 0's rows arrive on two
                    # separate DMAs (SWDGE + SP); later pairs share one.
                    if i == 0 or s == 0:
                        nc.tensor.ldweights(x_t[:, s, G:G + 1])
                    for c0 in range(0, nblk, PCH):
                        pt = ppool.tile([P, PCH], F32, name=f"pt{tidx}",
                                        tag="pt")
                        tn = None
                        if tidx >= PBUFS:
                            # Absorb the psum-buf-reuse WAR wait (eviction
                            # on DVE/ACT) on a ldweights reading one column
                            # of what that eviction wrote. The warm tile
                            # holds pool slot 0, so chunk tile t shares its
                            # buffer with chunk tile t-3 (not t-4).
                            prev_out = evs[tidx - (PBUFS - 1)][1]
                            tn = nc.tensor.ldweights(prev_out[:, 0:1])
                        for h in range(0, PCH, MM):
                            j0 = c0 + h
                            mm_l = nc.tensor.matmul(
                                pt[:, h:h + MM], wl,
                                x_t[:, s, G + j0:G + j0 + MM],
                                start=True, stop=False)
                            if tn is not None:
                                add_dep_helper(mm_l.ins, tn.ins, sync=False,
                                               reason="order abs before mm")
                                tn = None
                            last_mm = nc.tensor.matmul(
                                pt[:, h:h + MM], wm,
                                x_t[:, s, G - 1 + j0:G - 1 + j0 + MM],
                                start=False, stop=True)
                        out_ap = y_region(i, s)[:, c0:c0 + PCH]
                        # Alternate eviction engine per tile: DVE and ACT
                        # drain PSUM concurrently, so evictions never gate
                        # the PE's PSUM-buffer rotation.
                        if tidx % 2 == 0:
                            ev = nc.vector.tensor_copy(out_ap, pt[:])
                        else:
                            ev = nc.scalar.copy(out_ap, pt[:])
                        evs.append((ev, out_ap))
                        tidx += 1
                if i == 0:
                    # Pair 0's output ships over SWDGE while the input
                    # stream is still draining on the SP ring -- reads and
                    # writes overlap. Two gpsimd touches observe the DVE
                    # and ACT eviction semaphores first so the SWDGE DMA
                    # needs at most one wait.
                    gt1 = nc.gpsimd.tensor_copy(gscratch[:, 0:1],
                                                evs[6][1][:, 0:1])
                    gp_touch = nc.gpsimd.tensor_copy(gscratch[:, 1:2],
                                                     evs[7][1][:, 0:1])
                    add_dep_helper(gp_touch.ins, gt1.ins, sync=False,
                                   reason="order gp touches")
                    y0_dma = nc.gpsimd.dma_start(
                        y_d[0].rearrange("s p l -> p s l"), y_tiles[0][:])
                    yout.append(y0_dma)
                else:
                    dve_last = evs[8 * i + 6][1]
                    ship(y_d[i].rearrange("s p l -> p s l"), y_tiles[i][:],
                         dve_last)

            # Tail absorbers: observe every proc's final tick on single-wait
            # SP nops so the auto-generated kernel-tail drain needs no waits.
            tail_deps = [w_dma, x0a] + list(xin) + yout + [
                gp_touch, ship_abs[-1], evs[30][0], last_mm]
            prev = None
            for k, dep in enumerate(tail_deps):
                tn = nc.sync.nop(hint=f"tail{k}", nofuse=True)
                add_dep_helper(tn.ins, dep.ins, reason="tail drain absorb")
                if prev is not None:
                    add_dep_helper(tn.ins, prev.ins, sync=False,
                                   reason="tail chain order")
                prev = tn
    return nc


def kernel(inputs: np.ndarray) -> np.ndarray:
    global LAST_EXEC_TIME_NS, _nc_cache
    x = np.ascontiguousarray(inputs, dtype=np.float32)
    assert x.shape == (BATCH, T), x.shape
    # bf16 + block-transpose: sample 128j+p of row r -> xt[r, p, j]
    xb = x.astype(ml_dtypes.bfloat16).reshape(BATCH, NBLK, P)
    xt = np.zeros((BATCH, P, G + NBLK), dtype=ml_dtypes.bfloat16)
    xt[:, :, G:] = xb.transpose(0, 2, 1)
    w = _weights()

    if _nc_cache is None:
        _nc_cache = build_nc()
    nc = _nc_cache
    rows_per_core = BATCH // N_CORES
    in_maps = [
        {"x": xt[k * rows_per_core:(k + 1) * rows_per_core].reshape(
            PAIRS, SUB, P, G + NBLK),
         "w": w}
        for k in range(N_CORES)
    ]
    res = run_bass_kernel_spmd(nc, in_maps, list(range(N_CORES)))
    LAST_EXEC_TIME_NS = res.exec_time_ns
    out = np.empty((BATCH, T), dtype=np.float32)
    for k in range(N_CORES):
        yk = res.results[k]["y"].reshape(rows_per_core, P, NBLK)
        out[k * rows_per_core:(k + 1) * rows_per_core] = (
            yk.astype(np.float32).transpose(0, 2, 1).reshape(rows_per_core, T))
    return out
